# revision 23
# baseline (speedup 1.0000x reference)
"""Trainium2 Bass kernel for a GNN message-passing layer (C6).

Math (reference):
  h1[i,j,:] = concat(x_i, x_j, ef_ij) @ W1 + b1              (pre-relu hidden)
  msg       = relu(h1) @ W2 + b2
  agg[i]    = sum_j adj[i,j]>0 ? msg[i,j] : 0  / max(deg,1)
  out       = relu(concat(x, agg) @ U1 + ub1) @ U2 + ub2

Restructure: @W2 commutes with the masked sum; h1 = ef@W1e + x_j@W1j + a_i.

Sparsity compaction: the host gathers only real edges per node.  Nodes
are SORTED by degree per core and paired ascending, so the 24
lowest-degree pairs fit in a 512-wide single-PSUM-bank slot (2D AP) and
the rest use 576-wide two-bank slots (3D AP).  HW measurement: DVE is
~35% faster on single-bank 2D APs, ACT prefers wide 3D; slots are
statically interleaved [D2,D2,A3,D3]/[D2,A3,A3,D3] so both engines stay
busy continuously.  The output row permutation is undone at the end by
a 0/1 permutation matmul on the PE.

Per slot: two K=98 matmuls (moving fp8e4, stationary bf16) of width
W/2; ONE fused relu+bias+reduce instruction (in-place PSUM, accum_out):
  ACT:  activation(Relu, bias=a_i, accum_out)
  DVE:  tensor_scalar(max, -a_i, accum_out) with +W*a fixup in epilogue
The epilogue (agg@W2, update MLP, transpose, unpermute, DMA) is
software-pipelined across the next three loop bodies in reps mode.
"""

import numpy as np
import ml_dtypes
from contextlib import ExitStack

import concourse.bass as bass
import concourse.tile as tile
from concourse import bacc, mybir
from concourse.bass_utils import run_bass_kernel_spmd

N_CORES = 8
N, D, E, H = 1024, 32, 16, 64
RPC = N // N_CORES          # 128 source rows (i) per core
NPAIR = RPC // 2            # 64 slots (i-pairs) per core
BIG = 240.0                 # fits fp8e4 (max 448); |h1|+|a| << 240
F8 = ml_dtypes.float8_e4m3
BF16 = ml_dtypes.bfloat16

GROUP_PAIRS = 4             # slots per stage tile / pack DMA
KTOT = 98                   # ef(32) + padkill(2) + x(64)
NSTAGE = 4
NG = NPAIR // GROUP_PAIRS   # 16 groups

W2D, W3D = 512, 576
STAGE_W = 4 * W3D           # stage tile width bound (2304)

# Per-group slot patterns: (form, engine) with form in {2,3}, engine in
# {"A","D"}.  Alternating Ga/Gb keeps ACT/DVE load even over time:
#   Ga: ACT 1.14us, DVE 2.01us;  Gb: ACT 2.28us, DVE 1.44us.
_Ga = [(2, "D"), (2, "D"), (3, "A"), (3, "D")]
_Gb = [(2, "D"), (3, "A"), (3, "A"), (3, "D")]
GROUPS = [_Ga, _Gb] * (NG // 2)

SLOT_FORM, SLOT_ENG, SLOT_W, SLOT_OFF, GROUP_W = [], [], [], [], []
for g in range(NG):
    off = 0
    for q in range(GROUP_PAIRS):
        form, eng = GROUPS[g][q]
        w = W2D if form == 2 else W3D
        SLOT_FORM.append(form)
        SLOT_ENG.append(eng)
        SLOT_W.append(w)
        SLOT_OFF.append(off)
        off += w
    GROUP_W.append(off)

# host-side slot ordering: slot s gets the s-th sorted pair, but 2D slots
# must take the 24 lowest-degree pairs.  Build the mapping from "sorted
# pair rank" to slot id: 2D slots get ranks 0..n2-1 in slot order, 3D
# slots get the rest.
_slots_2d = [s for s in range(NPAIR) if SLOT_FORM[s] == 2]
_slots_3d = [s for s in range(NPAIR) if SLOT_FORM[s] == 3]
SLOT_RANK = [0] * NPAIR     # slot id -> sorted pair rank
for rank, s in enumerate(_slots_2d + _slots_3d):
    SLOT_RANK[s] = rank

_cache = {}


def _build(reps: int = 1, npairs: int = NPAIR, mode: str = "full"):
    nc = bacc.Bacc(
        "TRN2", target_bir_lowering=False, debug=False, num_devices=N_CORES
    )
    f32 = mybir.dt.float32
    bf = mybir.dt.bfloat16
    f8 = mybir.dt.float8e4

    t = {}
    def inp(name, shape, dt):
        t[name] = nc.dram_tensor(name, list(shape), dt, kind="ExternalInput").ap()

    inp("pack", (NG * KTOT, STAGE_W), f8)
    inp("statw", (KTOT, 128), bf)
    inp("permw", (RPC, RPC), f32)
    inp("c128", (128, 4 * NPAIR), f32)   # abias | nabias | rdeg | fixup
    inp("c64", (H, RPC + 3 * H + 2), f32)  # b2t | w2m | u2m | iden | ub1 | ub2
    inp("u1m", (D + H, H), f32)
    inp("xct", (D, RPC), f32)
    out = nc.dram_tensor("out", [RPC, H], f32, kind="ExternalOutput").ap()

    relu = mybir.ActivationFunctionType.Relu

    with tile.TileContext(nc) as tc:
        with ExitStack() as ctx:
            const = ctx.enter_context(tc.tile_pool(name="const", bufs=1))
            stpool = ctx.enter_context(tc.tile_pool(name="stage", bufs=1))
            psum = ctx.enter_context(tc.tile_pool(name="psum", bufs=3, space="PSUM"))
            psum2 = ctx.enter_context(tc.tile_pool(name="psum2", bufs=2, space="PSUM"))
            scr = ctx.enter_context(tc.tile_pool(name="scr", bufs=1))

            def load_const(name, shape, dt):
                sb = const.tile(list(shape), dt, tag=name)
                nc.gpsimd.dma_start(sb[:], t[name][:])
                return sb

            statw_sb = load_const("statw", (KTOT, 128), bf)
            permw_sb = load_const("permw", (RPC, RPC), f32)
            c128_sb = load_const("c128", (128, 4 * NPAIR), f32)
            c64_sb = load_const("c64", (H, RPC + 3 * H + 2), f32)
            u1_sb = load_const("u1m", (D + H, H), f32)
            abias_sb = c128_sb[:, 0 * NPAIR : 1 * NPAIR]
            nabias_sb = c128_sb[:, 1 * NPAIR : 2 * NPAIR]
            rdeg_sb = c128_sb[:, 2 * NPAIR : 3 * NPAIR]
            fixup_sb = c128_sb[:, 3 * NPAIR : 4 * NPAIR]
            b2t_sb = c64_sb[:, 0:RPC]
            w2_sb = c64_sb[:, RPC : RPC + H]
            u2_sb = c64_sb[:, RPC + H : RPC + 2 * H]
            iden_sb = c64_sb[:, RPC + 2 * H : RPC + 3 * H]
            ub1_sb = c64_sb[:, RPC + 3 * H : RPC + 3 * H + 1]
            ub2_sb = c64_sb[:, RPC + 3 * H + 1 : RPC + 3 * H + 2]

            combt = []
            for P in range(2):
                cb = const.tile([H + D, RPC], f32, tag=f"combt{P}")
                nc.gpsimd.dma_start(cb[H : H + D, :], t["xct"][:])
                combt.append(cb)

            stages = []
            for b in range(NSTAGE):
                st = stpool.tile([KTOT, STAGE_W], f8, tag=f"stage{b}")
                stages.append(st)

            acc_act, acc_dve = [], []
            for P in range(2):
                aa = const.tile([128, NPAIR], f32, tag=f"acc_act{P}")
                ad = const.tile([128, NPAIR], f32, tag=f"acc_dve{P}")
                nc.vector.memset(aa[:], 0.0)
                nc.vector.memset(ad[:], 0.0)
                acc_act.append(aa)
                acc_dve.append(ad)

            t4s, ssts, r1s, o2s, osbs, osb2s = [], [], [], [], [], []
            for P in range(2):
                t4 = scr.tile([128, NPAIR], f32, tag=f"t4_{P}")
                sst = scr.tile([H, NPAIR, 2], f32, tag=f"sst_{P}")
                r1 = scr.tile([H, RPC], f32, tag=f"r1_{P}")
                o2 = scr.tile([H, RPC], f32, tag=f"o2_{P}")
                osb = scr.tile([RPC, H], f32, tag=f"osb_{P}")
                osb2 = scr.tile([RPC, H], f32, tag=f"osb2_{P}")
                for tl in (t4, sst, r1, o2, osb, osb2):
                    nc.gpsimd.memset(tl[:], 0.0)
                t4s.append(t4); ssts.append(sst); r1s.append(r1)
                o2s.append(o2); osbs.append(osb); osb2s.append(osb2)

            warm = scr.tile([1, 1], f32, tag="warm")
            nc.vector.memset(warm[:], 0.0)
            warmo = scr.tile([1, 1], f32, tag="warmo")
            nc.scalar.activation(warmo[:], warm[:], relu)

            def emit_group(P, g):
                st = stages[g % NSTAGE]
                if mode != "nodma" or g < NSTAGE:
                    nc.sync.dma_start(
                        st[:, 0 : GROUP_W[g]],
                        t["pack"][g * KTOT : (g + 1) * KTOT, 0 : GROUP_W[g]],
                    )
                for q in range(GROUP_PAIRS):
                    s = g * GROUP_PAIRS + q
                    w = SLOT_W[s]
                    wh = w // 2
                    off = SLOT_OFF[s]
                    ps = psum.tile([128, 2, 512], f32, tag="ps")
                    for c in range(2):
                        dst = (
                            ps[:, 0, c * wh : (c + 1) * wh]
                            if SLOT_FORM[s] == 2
                            else ps[:, c, 0:wh]
                        )
                        nc.tensor.matmul(
                            dst,
                            lhsT=statw_sb[:],
                            rhs=st[:, off + c * wh : off + (c + 1) * wh],
                            start=True,
                            stop=True,
                        )
                    chunk = ps[:, 0, 0:w] if SLOT_FORM[s] == 2 else ps[:, :, 0:wh]
                    if mode == "noelt":
                        continue
                    eng = SLOT_ENG[s]
                    if mode == "alldve":
                        eng = "D"
                    elif mode == "allact":
                        eng = "A"
                    if eng == "A":
                        nc.scalar.activation(
                            chunk,
                            chunk,
                            relu,
                            bias=abias_sb[:, s : s + 1],
                            accum_out=acc_act[P][:, s : s + 1],
                        )
                    else:
                        nc.vector.tensor_scalar(
                            chunk,
                            chunk,
                            nabias_sb[:, s : s + 1],
                            0.0,
                            op0=mybir.AluOpType.max,
                            op1=mybir.AluOpType.add,
                            accum_out=acc_dve[P][:, s : s + 1],
                        )

            # ---- epilogue steps over parity X ----
            live = {}

            def s_t4(X):
                nc.gpsimd.tensor_add(t4s[X][:], acc_act[X][:], acc_dve[X][:])
                nc.gpsimd.tensor_add(t4s[X][:], t4s[X][:], fixup_sb[:])

            def s_sst(X):
                nc.gpsimd.tensor_mul(ssts[X][:, :, 0], t4s[X][0:H, :], rdeg_sb[0:H, :])
                nc.gpsimd.tensor_mul(ssts[X][:, :, 1], t4s[X][H:128, :], rdeg_sb[H:128, :])

            def s_aggmm(X):
                agp = psum2.tile([H, RPC], f32, tag="ep")
                nc.tensor.matmul(agp[:], lhsT=w2_sb[:], rhs=ssts[X][:], start=True, stop=True)
                live[("agp", X)] = agp

            def s_combt(X):
                nc.vector.tensor_add(combt[X][0:H, :], live[("agp", X)][:], b2t_sb[:])

            def s_u1mm(X):
                h2p = psum2.tile([H, RPC], f32, tag="ep")
                nc.tensor.matmul(h2p[:], lhsT=u1_sb[:], rhs=combt[X][:], start=True, stop=True)
                live[("h2p", X)] = h2p

            def s_r1(X):
                nc.scalar.activation(r1s[X][:], live[("h2p", X)][:], relu, bias=ub1_sb[:, 0:1])

            def s_u2mm(X):
                o2p = psum2.tile([H, RPC], f32, tag="ep")
                nc.tensor.matmul(o2p[:], lhsT=u2_sb[:], rhs=r1s[X][:], start=True, stop=True)
                live[("o2p", X)] = o2p

            def s_o2(X):
                nc.vector.tensor_scalar_add(o2s[X][:], live[("o2p", X)][:], ub2_sb[:, 0:1])

            def s_fin(X):
                fin = psum2.tile([RPC, H], f32, tag="ep")
                nc.tensor.transpose(fin[:], o2s[X][:], iden_sb[:])
                live[("fin", X)] = fin

            def s_osb(X):
                nc.vector.tensor_copy(osbs[X][:], live[("fin", X)][:])

            def s_pmm(X):
                # unpermute rows: out[true] = sum_pos perm[pos,true]*osb[pos]
                pfin = psum2.tile([RPC, H], f32, tag="ep")
                nc.tensor.matmul(pfin[:], lhsT=permw_sb[:], rhs=osbs[X][:], start=True, stop=True)
                live[("pfin", X)] = pfin

            def s_pcopy(X):
                nc.vector.tensor_copy(osb2s[X][:], live[("pfin", X)][:])

            def s_out(X):
                nc.sync.dma_start(out[:], osb2s[X][:])

            SEQ = [s_t4, s_sst, s_aggmm, s_combt, s_u1mm, s_r1,
                   s_u2mm, s_o2, s_fin, s_osb, s_pmm, s_pcopy, s_out]

            if reps == 1:
                for g in range(NG):
                    emit_group(0, g)
                for step in SEQ:
                    step(0)
            else:
                assert reps % 2 == 0
                SCHED = {
                    0: [(s_t4, "Q")],
                    1: [(s_sst, "Q")],
                    2: [(s_aggmm, "P")],
                    4: [(s_combt, "P")],
                    6: [(s_u1mm, "P")],
                    8: [(s_r1, "P")],
                    9: [(s_u2mm, "Q")],
                    10: [(s_o2, "Q")],
                    11: [(s_fin, "Q")],
                    12: [(s_osb, "Q")],
                    13: [(s_pmm, "Q")],
                    14: [(s_pcopy, "Q")],
                    15: [(s_out, "Q")],
                }
                with tc.For_i(0, reps // 2, 1):
                    for P in (0, 1):
                        Q = 1 - P
                        for g in range(NG):
                            emit_group(P, g)
                            for step, par in SCHED.get(g, []):
                                step(P if par == "P" else Q)

    nc.compile()
    return nc


def _prep_maps(node_features, edge_features, adjacency, W1, b1, W2, b2, U1, ub1, U2, ub2):
    nf = np.ascontiguousarray(node_features, np.float32)
    ef = np.ascontiguousarray(edge_features, np.float32)
    adj = np.asarray(adjacency)
    W1 = np.asarray(W1, np.float32)
    b1 = np.asarray(b1, np.float32)

    W1i, W1j, W1e = W1[0:D], W1[D : 2 * D], W1[2 * D :]
    A = nf @ W1i + b1[None, :]              # (N, H) fp32
    mask = adj > 0
    deg = adj.sum(axis=1).astype(np.float32)
    cnt = mask.sum(axis=1).astype(np.float32)
    degc = np.where(deg == 0, 1.0, deg)
    ni = mask.sum(axis=1)

    # compacted edge order: real-edge j's first (ascending), then the rest
    order = np.argsort(~mask, axis=1, kind="stable")

    stat = np.zeros((KTOT, 128), np.float32)
    stat[0:16, 0:64] = W1e
    stat[16:32, 64:128] = W1e
    stat[32, 0:64] = 1.0
    stat[33, 64:128] = 1.0
    stat[34:66, 0:64] = W1j
    stat[66:98, 64:128] = W1j

    ef3 = ef.reshape(N, N, E)
    ndve = np.array([0.0 if SLOT_ENG[s] == "A" else 1.0 for s in range(NPAIR)],
                    np.float32)
    slot_w = np.array(SLOT_W, np.float32)

    maps = []
    for core in range(N_CORES):
        i0 = core * RPC
        sl = slice(i0, i0 + RPC)
        dcore = ni[sl]
        # sort rows by degree ascending; rank r pair = rows (2r, 2r+1)
        sidx = np.argsort(dcore, kind="stable")
        # slot s takes sorted-pair rank SLOT_RANK[s]
        rows0 = np.empty(NPAIR, np.int64)
        rows1 = np.empty(NPAIR, np.int64)
        for s in range(NPAIR):
            r = SLOT_RANK[s]
            rows0[s] = sidx[2 * r]
            rows1[s] = sidx[2 * r + 1]
        pair_max = np.maximum(dcore[rows0], dcore[rows1])
        assert all(pair_max[s] <= SLOT_W[s] for s in range(NPAIR)), (
            core, [(s, int(pair_max[s]), SLOT_W[s]) for s in range(NPAIR)
                   if pair_max[s] > SLOT_W[s]])

        # sorted position -> true local row, for constants and permw
        pos_row = np.empty(RPC, np.int64)
        for s in range(NPAIR):
            pos_row[2 * s] = rows0[s]
            pos_row[2 * s + 1] = rows1[s]

        pk = np.zeros((NG, KTOT, STAGE_W), np.float32)
        for s in range(NPAIR):
            g, off, w = s // GROUP_PAIRS, SLOT_OFF[s], SLOT_W[s]
            for half, row in ((0, rows0[s]), (1, rows1[s])):
                gi = i0 + row
                J = order[gi, :w]
                n = int(ni[gi])
                efg = ef3[gi, J]                       # (w, 16)
                xg = nf[J]                             # (w, 32)
                rb = 0 if half == 0 else 16
                pk[g, rb : rb + 16, off : off + w] = efg.T
                pk[g, 32 + half, off : off + w] = np.where(
                    np.arange(w) < n, 0.0, -BIG)
                xb = 34 if half == 0 else 66
                pk[g, xb : xb + 32, off : off + w] = xg.T

        Ac = A[sl]
        abias_c = np.empty((128, NPAIR), np.float32)
        abias_c[0:64] = Ac[rows0].T
        abias_c[64:128] = Ac[rows1].T
        fixup_c = abias_c * (slot_w * ndve)[None, :]

        rdc = (1.0 / degc[sl]).astype(np.float32)
        rdeg_c = np.empty((128, NPAIR), np.float32)
        rdeg_c[0:64] = np.broadcast_to(rdc[rows0][None, :], (64, NPAIR))
        rdeg_c[64:128] = np.broadcast_to(rdc[rows1][None, :], (64, NPAIR))

        scale = (cnt[sl] / degc[sl])
        b2t_c = np.asarray(b2, np.float32)[:, None] * scale[pos_row][None, :]

        permw = np.zeros((RPC, RPC), np.float32)
        permw[np.arange(RPC), pos_row] = 1.0

        c128 = np.concatenate(
            [abias_c, -abias_c, rdeg_c, fixup_c], axis=1
        ).astype(np.float32)
        c64 = np.concatenate(
            [
                np.ascontiguousarray(b2t_c, np.float32),
                np.asarray(W2, np.float32),
                np.asarray(U2, np.float32),
                np.eye(H, dtype=np.float32),
                np.asarray(ub1, np.float32).reshape(H, 1),
                np.asarray(ub2, np.float32).reshape(H, 1),
            ],
            axis=1,
        ).astype(np.float32)
        maps.append(
            {
                "pack": pk.reshape(NG * KTOT, STAGE_W).astype(F8),
                "statw": stat.astype(BF16),
                "permw": permw,
                "c128": np.ascontiguousarray(c128),
                "c64": np.ascontiguousarray(c64),
                "u1m": np.concatenate(
                    [np.asarray(U1, np.float32)[D:], np.asarray(U1, np.float32)[:D]]
                ),
                "xct": np.ascontiguousarray(nf[sl][pos_row].T, np.float32),
            }
        )
    return maps


def kernel(**inputs) -> np.ndarray:
    if "nc" not in _cache:
        _cache["nc"] = _build()
    nc = _cache["nc"]
    maps = _prep_maps(
        inputs["node_features"],
        inputs["edge_features"],
        inputs["adjacency"],
        inputs["W1"],
        inputs["b1"],
        inputs["W2"],
        inputs["b2"],
        inputs["U1"],
        inputs["ub1"],
        inputs["U2"],
        inputs["ub2"],
    )
    res = run_bass_kernel_spmd(nc, maps, list(range(N_CORES)))
    outs = [np.asarray(res.results[i]["out"], np.float32) for i in range(N_CORES)]
    return np.concatenate(outs, axis=0)
